# revision 10
# baseline (speedup 1.0000x reference)
"""Sliding-window causal self-attention (GQA + RoPE + RMS-norm + value-embedding
gate) for Trainium2, sharded over 8 NeuronCores.

Sharding: sequence-parallel. (batch=2) x (4 sequence chunks of 1024) = 8 shards.
Each core computes attention for its own 1024 query rows. Window size = 1024 and
chunk size = 1024, so each core only needs K/V for its own chunk plus the
previous 1024 positions (halo). K/V (+rope/rms/gate) are recomputed locally for
the halo instead of communicated -> zero collectives. Chunk-0 shards get a
zero-padded halo; padded keys produce k=0 => exp(score)=0+... exp(0)=1 which is
corrected exactly by subtracting the per-row pad count from the softmax
denominator (padded v rows are 0 so the numerator is untouched).

Key kernel trick: scores are computed pre-transposed (s[k,q] via lhsT=kT,
rhs=qT) so the exp output (bf16) is directly the lhsT of the PV matmul, and V is
augmented with a ones column so the PV matmul emits y[q, 0:128] AND the softmax
denominator Z = y[q, 128] in one accumulation group. Normalization 1/Z is then a
native per-partition tensor_scalar in the natural q-layout.
"""

import math
import sys

import numpy as np

sys.path.insert(0, "/opt/trn_rl_repo")

import ml_dtypes

import concourse.bass as bass
import concourse.bacc as bacc
import concourse.tile as tile
from concourse import mybir
from concourse import bass_utils

BF16 = ml_dtypes.bfloat16
F32 = np.float32

B, T, C = 2, 4096, 1024
H, HKV, D = 8, 2, 128
REP = H // HKV
WIN = 1024
RCHUNK = 1024          # own rows per core
E = 2048               # ext rows (halo + own)
NRT = E // 128         # 16 ext row tiles
NQT = RCHUNK // 128    # 8 q tiles
NKC = 9                # k chunks per q tile
NCT = C // 128         # 8 contraction tiles
EPS = float(np.finfo(np.float32).eps)
SCALE = 1.0 / math.sqrt(D)
NEG = -1.0e30

dt = mybir.dt
AF = mybir.ActivationFunctionType
ALU = mybir.AluOpType
AX = mybir.AxisListType


def _bcast(ap, n, axis_pos=1):
    """Insert a 0-stride dim of size n into an AP at free-axis position."""
    new_ap = list(ap.ap)
    new_ap.insert(axis_pos, [0, n])
    return bass.AP(tensor=ap.tensor, offset=ap.offset, ap=new_ap)


def build_nc():
    nc = bacc.Bacc("TRN2", target_bir_lowering=False, debug=False)

    xT_d = nc.dram_tensor("xT", [C, E], dt.bfloat16, kind="ExternalInput").ap()
    wq_d = nc.dram_tensor("wq", [C, C], dt.bfloat16, kind="ExternalInput").ap()
    wkv_d = nc.dram_tensor("wkv", [C, 512], dt.bfloat16, kind="ExternalInput").ap()
    wo_d = nc.dram_tensor("wo", [C, C], dt.bfloat16, kind="ExternalInput").ap()
    wg_d = nc.dram_tensor("wg", [32, HKV], dt.bfloat16, kind="ExternalInput").ap()
    ve_d = nc.dram_tensor("ve2", [E, HKV * D], dt.bfloat16, kind="ExternalInput").ap()
    cs_d = nc.dram_tensor("cs", [E, 128], dt.bfloat16, kind="ExternalInput").ap()
    tri_d = nc.dram_tensor("tri", [128, 2 * 128], dt.float32, kind="ExternalInput").ap()
    npad_d = nc.dram_tensor("npad", [128, NQT], dt.float32, kind="ExternalInput").ap()
    id_d = nc.dram_tensor("ident", [128, 128], dt.bfloat16, kind="ExternalInput").ap()
    out_d = nc.dram_tensor("out", [RCHUNK, C], dt.float32, kind="ExternalOutput").ap()

    with tile.TileContext(nc) as tc:
        _body(tc, xT_d, wq_d, wkv_d, wo_d, wg_d, ve_d, cs_d, tri_d, npad_d, id_d,
              out_d)
    nc.compile()
    return nc


def _body(tc, xT_d, wq_d, wkv_d, wo_d, wg_d, ve_d, cs_d, tri_d, npad_d, id_d,
          out_d):
    nc = tc.nc
    from contextlib import ExitStack

    with ExitStack() as ctx:
        const = ctx.enter_context(tc.tile_pool(name="const", bufs=1))
        persist = ctx.enter_context(tc.tile_pool(name="persist", bufs=1))
        work = ctx.enter_context(tc.tile_pool(name="work", bufs=3))

        # ---- constants / persistent SBUF ----
        wq_sb = const.tile([128, NCT, C], dt.bfloat16)
        nc.sync.dma_start(out=wq_sb, in_=wq_d.rearrange("(a p) n -> p a n", p=128))
        wkv_sb = const.tile([128, NCT, 512], dt.bfloat16)
        nc.sync.dma_start(out=wkv_sb, in_=wkv_d.rearrange("(a p) n -> p a n", p=128))
        wo_sb = const.tile([128, NCT, C], dt.bfloat16)
        nc.sync.dma_start(out=wo_sb, in_=wo_d.rearrange("(a p) n -> p a n", p=128))
        wg_sb = const.tile([32, HKV], dt.bfloat16)
        nc.sync.dma_start(out=wg_sb, in_=wg_d)
        ve_sb = const.tile([128, NRT, HKV * D], dt.bfloat16)
        nc.sync.dma_start(out=ve_sb, in_=ve_d.rearrange("(a p) n -> p a n", p=128))
        cs_sb = const.tile([128, NRT, 128], dt.bfloat16)
        nc.sync.dma_start(out=cs_sb, in_=cs_d.rearrange("(a p) n -> p a n", p=128))
        tri_sb = const.tile([128, 2, 128], dt.float32)
        nc.sync.dma_start(out=tri_sb, in_=tri_d.rearrange("p (a n) -> p a n", a=2))
        npad_sb = const.tile([128, NQT], dt.float32)
        nc.sync.dma_start(out=npad_sb, in_=npad_d)
        id_sb = const.tile([128, 128], dt.bfloat16)
        nc.sync.dma_start(out=id_sb, in_=id_d)

        kT_sb = persist.tile([128, HKV, NRT, 128], dt.bfloat16)   # [d, kvh, g, k]
        qT_sb = persist.tile([128, H, NQT, 128], dt.bfloat16)     # [d, h, qt, q]
        v_sb = persist.tile([128, NRT, HKV, 129], dt.bfloat16)    # [k, g, kvh, d|1]
        yN_sb = persist.tile([128, NQT, H, 128], dt.bfloat16)     # [q, qt, h, d]
        krot_sb = persist.tile([128, NRT, HKV * D], dt.bfloat16)  # roped k (pre-norm)
        qrot_sb = persist.tile([128, NQT, C], dt.bfloat16)        # roped q (pre-norm)
        msk_sb = persist.tile([128, NRT, HKV], dt.float32)
        msq_sb = persist.tile([128, NQT, H], dt.float32)
        gate_sb = persist.tile([128, NRT, HKV], dt.float32)

        nc.vector.memset(v_sb[:, :, :, 128:129], 1.0)

        # ================= phase B1: projections + rope + stats =================
        with tc.tile_pool(name="xpool", bufs=1) as xpool, \
             tc.tile_pool(name="kvps", bufs=2, space="PSUM") as kvps, \
             tc.tile_pool(name="qps", bufs=1, space="PSUM") as qps, \
             tc.tile_pool(name="gps", bufs=1, space="PSUM") as gps:

            xT_sb = xpool.tile([128, NCT, E], dt.bfloat16)
            for ct in range(NCT):
                nc.sync.dma_start(
                    out=xT_sb[:, ct, :],
                    in_=xT_d.rearrange("(a p) n -> p a n", p=128)[:, ct, :])

            g_psum = gps.tile([128, NRT * HKV], dt.float32)

            for rt in range(NRT):
                rs = bass.ts(rt, 128)
                # --- kv projection: psum [128 rows, 256 k | 256 v] ---
                kv = kvps.tile([128, 512], dt.float32, tag="kv")
                for ct in range(NCT):
                    nc.tensor.matmul(kv, lhsT=xT_sb[:, ct, rs], rhs=wkv_sb[:, ct, :],
                                     start=(ct == 0), stop=(ct == NCT - 1))
                # --- gate matmul (K=32) ---
                nc.tensor.matmul(g_psum[:, bass.ts(rt, HKV)],
                                 lhsT=xT_sb[0:32, 0, rs], rhs=wg_sb,
                                 start=True, stop=True)
                # --- v raw copy (gate applied later in B2) ---
                nc.scalar.copy(out=v_sb[:, rt, :, 0:128], in_=kv[:, 256:512]
                               .rearrange("p (a n) -> p a n", a=HKV))
                # --- k rope (bf16, from psum via cast copy) ---
                kraw = work.tile([128, HKV * D], dt.bfloat16, tag="kraw")
                nc.scalar.copy(out=kraw, in_=kv[:, 0:256])
                k3 = kraw.rearrange("p (a n) -> p a n", a=HKV)
                kr3 = krot_sb[:, rt, :].rearrange("p (a n) -> p a n", a=HKV)
                cosb = _bcast(cs_sb[:, rt, 0:64], HKV)
                sinb = _bcast(cs_sb[:, rt, 64:128], HKV)
                t1 = work.tile([128, HKV, 64], dt.bfloat16, tag="t1")
                t2 = work.tile([128, HKV, 64], dt.bfloat16, tag="t2")
                nc.vector.tensor_mul(t1, k3[:, :, 0:64], cosb)
                nc.vector.tensor_mul(t2, k3[:, :, 64:128], sinb)
                nc.vector.tensor_add(kr3[:, :, 0:64], t1, t2)
                nc.vector.tensor_mul(t1, k3[:, :, 64:128], cosb)
                nc.vector.tensor_mul(t2, k3[:, :, 0:64], sinb)
                nc.vector.tensor_sub(kr3[:, :, 64:128], t1, t2)
                # --- k rms stats ---
                ksq = work.tile([128, HKV * D], dt.bfloat16, tag="kraw2")
                nc.vector.tensor_mul(ksq, krot_sb[:, rt, :], krot_sb[:, rt, :])
                nc.vector.tensor_reduce(
                    out=msk_sb[:, rt, :],
                    in_=ksq.rearrange("p (a n) -> p a n", a=HKV),
                    axis=AX.X, op=ALU.add)

                # --- q path (own rows only) ---
                if rt >= NRT - NQT:
                    qt = rt - (NRT - NQT)
                    qp = qps.tile([128, C], dt.float32, tag="q")
                    for half in range(2):
                        o = qp[:, bass.ts(half, 512)]
                        for ct in range(NCT):
                            nc.tensor.matmul(
                                o, lhsT=xT_sb[:, ct, rs],
                                rhs=wq_sb[:, ct, bass.ts(half, 512)],
                                start=(ct == 0), stop=(ct == NCT - 1))
                    qraw = work.tile([128, C], dt.bfloat16, tag="qraw")
                    nc.scalar.copy(out=qraw, in_=qp)
                    q3 = qraw.rearrange("p (a n) -> p a n", a=H)
                    qr3 = qrot_sb[:, qt, :].rearrange("p (a n) -> p a n", a=H)
                    cosbq = _bcast(cs_sb[:, rt, 0:64], H)
                    sinbq = _bcast(cs_sb[:, rt, 64:128], H)
                    u1 = work.tile([128, H, 64], dt.bfloat16, tag="u1")
                    u2 = work.tile([128, H, 64], dt.bfloat16, tag="u2")
                    nc.vector.tensor_mul(u1, q3[:, :, 0:64], cosbq)
                    nc.vector.tensor_mul(u2, q3[:, :, 64:128], sinbq)
                    nc.vector.tensor_add(qr3[:, :, 0:64], u1, u2)
                    nc.vector.tensor_mul(u1, q3[:, :, 64:128], cosbq)
                    nc.vector.tensor_mul(u2, q3[:, :, 0:64], sinbq)
                    nc.vector.tensor_sub(qr3[:, :, 64:128], u1, u2)
                    qsq = work.tile([128, C], dt.bfloat16, tag="qraw2")
                    nc.vector.tensor_mul(qsq, qrot_sb[:, qt, :], qrot_sb[:, qt, :])
                    nc.vector.tensor_reduce(
                        out=msq_sb[:, qt, :],
                        in_=qsq.rearrange("p (a n) -> p a n", a=H),
                        axis=AX.X, op=ALU.add)

            # one sigmoid for all row tiles (single ACT table load)
            nc.scalar.activation(
                out=gate_sb.rearrange("p a n -> p (a n)"),
                in_=g_psum, func=AF.Sigmoid)

        # ================= phase B2: normalize + transposes =================
        with tc.tile_pool(name="tp", bufs=2, space="PSUM") as tp:
            eps_sb = const.tile([128, 1], dt.float32)
            nc.vector.memset(eps_sb, EPS)
            msk_f = msk_sb.rearrange("p a n -> p (a n)")
            msq_f = msq_sb.rearrange("p a n -> p (a n)")
            nc.scalar.activation(out=msk_f, in_=msk_f, func=AF.Sqrt,
                                 bias=eps_sb, scale=1.0 / D)
            nc.scalar.activation(out=msq_f, in_=msq_f, func=AF.Sqrt,
                                 bias=eps_sb, scale=1.0 / D)
            nc.vector.reciprocal(out=msk_f, in_=msk_f)
            nc.vector.reciprocal(out=msq_f, in_=msq_f)

            for rt in range(NRT):
                kr3 = krot_sb[:, rt, :].rearrange("p (a n) -> p a n", a=HKV)
                for kvh in range(HKV):
                    # normalize k in place, then transpose -> kT
                    nc.vector.tensor_scalar_mul(
                        kr3[:, kvh, :], kr3[:, kvh, :], msk_sb[:, rt, kvh:kvh + 1])
                    ktp = tp.tile([128, 128], dt.bfloat16, tag="tp")
                    nc.tensor.transpose(ktp, kr3[:, kvh, :], id_sb)
                    nc.scalar.copy(out=kT_sb[:, kvh, rt, :], in_=ktp)
                    # v = v_raw + gate * ve
                    nc.vector.scalar_tensor_tensor(
                        out=v_sb[:, rt, kvh, 0:128],
                        in0=ve_sb[:, rt, bass.ts(kvh, 128)],
                        scalar=gate_sb[:, rt, kvh:kvh + 1],
                        in1=v_sb[:, rt, kvh, 0:128],
                        op0=ALU.mult, op1=ALU.add)

            for qt in range(NQT):
                qr3 = qrot_sb[:, qt, :].rearrange("p (a n) -> p a n", a=H)
                for h in range(H):
                    nc.vector.tensor_scalar(
                        out=qr3[:, h, :], in0=qr3[:, h, :],
                        scalar1=msq_sb[:, qt, h:h + 1], scalar2=SCALE,
                        op0=ALU.mult, op1=ALU.mult)
                    qtp = tp.tile([128, 128], dt.bfloat16, tag="tp")
                    nc.tensor.transpose(qtp, qr3[:, h, :], id_sb)
                    nc.scalar.copy(out=qT_sb[:, h, qt, :], in_=qtp)

        # ================= phase C: attention =================
        with tc.tile_pool(name="sps", bufs=2, space="PSUM") as sps, \
             tc.tile_pool(name="yps", bufs=2, space="PSUM") as yps:
            for h in range(H):
                kvh = h // REP
                for qt in range(NQT):
                    s = sps.tile([128, NKC, 128], dt.float32, tag="s")
                    for kc in range(NKC):
                        g = qt + kc
                        nc.tensor.matmul(s[:, kc, :], lhsT=kT_sb[:, kvh, g, :],
                                         rhs=qT_sb[:, h, qt, :],
                                         start=True, stop=True)
                    nc.vector.tensor_add(s[:, 0, :], s[:, 0, :], tri_sb[:, 0, :])
                    nc.vector.tensor_add(s[:, NKC - 1, :], s[:, NKC - 1, :],
                                         tri_sb[:, 1, :])
                    p = work.tile([128, NKC, 128], dt.bfloat16, tag="p")
                    nc.scalar.activation(
                        out=p.rearrange("p a n -> p (a n)"),
                        in_=s.rearrange("p a n -> p (a n)"), func=AF.Exp)
                    y = yps.tile([128, 129], dt.float32, tag="y")
                    for kc in range(NKC):
                        g = qt + kc
                        nc.tensor.matmul(y, lhsT=p[:, kc, :],
                                         rhs=v_sb[:, g, kvh, :],
                                         start=(kc == 0), stop=(kc == NKC - 1))
                    z = work.tile([128, 1], dt.float32, tag="z")
                    nc.vector.tensor_sub(z, y[:, 128:129], npad_sb[:, qt:qt + 1])
                    nc.vector.reciprocal(out=z, in_=z)
                    nc.vector.tensor_scalar_mul(yN_sb[:, qt, h, :], y[:, 0:128], z)

        # ================= phase D: output projection =================
        with tc.tile_pool(name="tp2", bufs=2, space="PSUM") as tp2, \
             tc.tile_pool(name="ops", bufs=2, space="PSUM") as ops:
            for qt in range(NQT):
                yT = work.tile([128, H, 128], dt.bfloat16, tag="yT")
                for h in range(H):
                    ytp = tp2.tile([128, 128], dt.bfloat16, tag="ytp")
                    nc.tensor.transpose(ytp, yN_sb[:, qt, h, :], id_sb)
                    nc.scalar.copy(out=yT[:, h, :], in_=ytp)
                for half in range(2):
                    o = ops.tile([128, 512], dt.float32, tag="o")
                    for h in range(H):
                        nc.tensor.matmul(o, lhsT=yT[:, h, :],
                                         rhs=wo_sb[:, h, bass.ts(half, 512)],
                                         start=(h == 0), stop=(h == H - 1))
                    osb = work.tile([128, 512], dt.float32, tag="osb")
                    nc.scalar.copy(out=osb, in_=o)
                    nc.sync.dma_start(
                        out=out_d[bass.ts(qt, 128), bass.ts(half, 512)], in_=osb)


# ---------------------------------------------------------------------------
# host side
# ---------------------------------------------------------------------------

def make_in_maps(x, ve, cos, sin, Wq, Wk, Wv, Wproj, Wg):
    """Build the 8 per-core input dicts (numpy, host-side prep)."""
    x = np.asarray(x, F32)
    ve = np.asarray(ve, F32)
    cos = np.asarray(cos, F32).reshape(T, 64)
    sin = np.asarray(sin, F32).reshape(T, 64)
    Wq = np.asarray(Wq, F32)
    Wk = np.asarray(Wk, F32)
    Wv = np.asarray(Wv, F32)
    Wproj = np.asarray(Wproj, F32)
    Wg = np.asarray(Wg, F32)

    wq = Wq.astype(BF16)
    wkv = np.concatenate([Wk, Wv], axis=1).astype(BF16)
    wo = Wproj.astype(BF16)
    wg = Wg.astype(BF16)
    ident = np.eye(128, dtype=BF16)

    # triangular masks in [k, q] layout
    kk = np.arange(128)[:, None]
    qq = np.arange(128)[None, :]
    tri = np.zeros((128, 2, 128), F32)
    tri[:, 0, :] = np.where(kk < qq, NEG, 0.0)   # LEFT chunk (kc=0)
    tri[:, 1, :] = np.where(kk > qq, NEG, 0.0)   # DIAG chunk (kc=8)
    tri = tri.reshape(128, 256)

    in_maps = []
    for c in range(8):
        b, ck = divmod(c, 4)
        t0 = ck * RCHUNK
        es = t0 - WIN  # ext start (may be negative for chunk 0)
        pad = max(0, -es)

        def ext(a, fill_shape):
            out = np.zeros((E,) + fill_shape, F32)
            out[pad:] = a[es + pad: t0 + RCHUNK]
            return out

        x_e = ext(x[b], (C,))
        ve_e = ext(ve[b], (HKV * D,))
        cos_e = ext(cos, (64,))
        sin_e = ext(sin, (64,))

        npad = np.zeros((128, NQT), F32)
        if pad:
            kc = np.arange(NKC)[:, None]
            kl = np.arange(128)[None, :]
            r = np.arange(128)
            for qt in range(NQT):
                extpos = 128 * (qt + kc) + kl          # [9, 128]
                is_pad = extpos < pad
                for ri in r:
                    tri_ok = np.ones((NKC, 128), bool)
                    tri_ok[0] = kl[0] >= ri
                    tri_ok[NKC - 1] = kl[0] <= ri
                    npad[ri, qt] = np.sum(tri_ok & is_pad)

        in_maps.append({
            "xT": np.ascontiguousarray(x_e.T).astype(BF16),
            "wq": wq, "wkv": wkv, "wo": wo, "wg": wg,
            "ve2": (2.0 * ve_e).astype(BF16),
            "cs": np.concatenate([cos_e, sin_e], axis=1).astype(BF16),
            "tri": tri, "npad": npad, "ident": ident,
        })
    return in_maps


_NC_CACHE = None


def kernel(x, ve, cos, sin, Wq, Wk, Wv, Wproj, Wg, window_size):
    assert int(window_size) == WIN
    global _NC_CACHE
    if _NC_CACHE is None:
        _NC_CACHE = build_nc()
    nc = _NC_CACHE
    in_maps = make_in_maps(x, ve, cos, sin, Wq, Wk, Wv, Wproj, Wg)
    res = bass_utils.run_bass_kernel_spmd(nc, in_maps, core_ids=list(range(8)))
    out = np.zeros((B, T, C), F32)
    for c in range(8):
        b, ck = divmod(c, 4)
        out[b, ck * RCHUNK:(ck + 1) * RCHUNK] = res.results[c]["out"]
    return out
